# revision 38
# baseline (speedup 1.0000x reference)
"""Trainium2 Bass kernel for blocked (non-overlapping window) attention.

Reference computation (per batch b):
    q = Wq @ x1 ; k = Wk @ x1 ; v = Wv @ x1          (1x1 convs, biases)
    split L into blocks of 64; per block: softmax((q^T k)/sqrt(C) masked) @ v^T
    h = relu(attn); out = Wo @ h + bo

Sharding: sequence-parallel over L: each of 8 cores gets a contiguous
L/8 = 2048 slice for all 4 batches; small conv weights replicated.

Numerics: matmuls bf16, f32 psum accumulation. 1/sqrt(C) is folded into
Wq host-side (exact: power of two). Two 64-blocks are processed per
128-partition pair; two pairs (256 l-positions) per attention step.
Softmax skips max-subtraction; the cross-block quadrants of exp(scores)
are zeroed by two vector memsets before the row-sum (no additive mask
matmul needed when the user mask is all ones). Output is stored bf16
and upconverted host-side.

Fast path assumes the zero biases / all-ones mask that setup_inputs()
produces; general inputs fall back to a variant graph with bias
epilogues and per-(batch,step) additive -30000 mask matmuls.

Schedule notes (the big wins over a naive phase-by-phase loop):
 - batch b+1's q/k projection groups (pure PE work) are interleaved
   into batch b's attention steps, so the scalar/vector-heavy softmax
   epilogues always overlap PE-heavy projection segments;
 - 3-stage software pipeline inside attention: transpose(u) trails
   scores(u) by two PE packets, hiding the cross-engine
   exp->reduce->recip->normalize chain;
 - output projection is emitted as per-(tile, o-chunk) pieces
   sprinkled between steps, stores stream on the sync queue
   immediately (bf16, one descriptor per piece);
 - epilogue casts alternate scalar/vector (gpsimd has no PSUM port and
   its tensor ops are ~8x slower - only DMA issue is cheap there);
 - psum banks: 2 shared (q/k/v/out) + 3 scores (pipeline depth) +
   1 transpose + 2 attention = 8 exactly.
"""

import sys

sys.path.insert(0, "/opt/trn_rl_repo")

import numpy as np
import ml_dtypes

B = 4
C_IN = 512
L = 16384
CR = 256           # reduced (q/k/v) channels
BL = 64            # attention block
N_CORES = 8
LS = L // N_CORES  # 2048 per-core sequence shard
NT = LS // 512     # 4 free-dim tiles of 512
KC = C_IN // 128   # 4 contraction chunks for q/k/v projections
MC = CR // 128     # 2 chunks of reduced channels
OC = C_IN // 128   # 4 chunks of output channels
NPAIR = LS // 128  # 16 block-pairs per batch per core
NSTEP = NPAIR // 2 # 8 attention steps (2 pairs each) per batch
NEGM = -30000.0
QSCALE = 1.0 / 16.0  # 1/sqrt(C_RED)

_CACHE = {}


def _build_graph(has_bias, has_mask):
    import concourse.bass as bass
    import concourse.tile as tile
    from concourse import bacc, mybir

    f32 = mybir.dt.float32
    bf16 = mybir.dt.bfloat16
    AF = mybir.ActivationFunctionType

    nc = bacc.Bacc(None, target_bir_lowering=False)

    x1_e = nc.declare_dram_parameter("x1s", [B, NT, 128, KC, 512], bf16, isOutput=False)
    wq_e = nc.declare_dram_parameter("wq", [128, KC, CR], bf16, isOutput=False)
    wk_e = nc.declare_dram_parameter("wk", [128, KC, CR], bf16, isOutput=False)
    wv_e = nc.declare_dram_parameter("wv", [128, KC, CR], bf16, isOutput=False)
    wo_e = nc.declare_dram_parameter("wo", [128, MC, C_IN], bf16, isOutput=False)
    id_e = nc.declare_dram_parameter("ident", [128, 128], bf16, isOutput=False)
    if has_mask:
        on_e = nc.declare_dram_parameter("onesbd", [2, 128], bf16, isOutput=False)
        md_e = nc.declare_dram_parameter("madd2", [B, 2, NSTEP, 256], bf16, isOutput=False)
    if has_bias:
        # packed per-channel biases: cols [0:2]=bq*QSCALE, [2:4]=bk, [4:6]=bv, [6:10]=bo
        bia_e = nc.declare_dram_parameter("biases", [128, 2 * MC + 2 + OC], f32, isOutput=False)
    out_e = nc.declare_dram_parameter("out", [B, NT, OC, 128, 512], bf16, isOutput=True)

    PS = bass.MemorySpace.PSUM

    with tile.TileContext(nc) as tc:
        with (
            tc.tile_pool(name="const", bufs=1) as constp,
            tc.tile_pool(name="x1p", bufs=2) as x1p,
            tc.tile_pool(name="qkp", bufs=32) as qkp,
            tc.tile_pool(name="vtp", bufs=16) as vtp,
            tc.tile_pool(name="hp", bufs=6) as hp,
            tc.tile_pool(name="outp", bufs=4) as outp,
            tc.tile_pool(name="smp", bufs=4) as smp,
            tc.tile_pool(name="psA", bufs=3, space=PS) as psA,
            tc.tile_pool(name="psS", bufs=2, space=PS) as psS,
            tc.tile_pool(name="psT", bufs=1, space=PS) as psT,
            tc.tile_pool(name="psAt", bufs=2, space=PS) as psAt,
        ):
            # ---- constants (DMA'd on the scalar queue, ordered by first use;
            # x1 tile loads ride the sync queue in parallel) ----
            wq_sb = constp.tile([128, KC, CR], bf16, tag="wq")
            wk_sb = constp.tile([128, KC, CR], bf16, tag="wk")
            # critical-path order: x1 t0 halves lead the sync queue (below);
            # wq's first m-chunk leads the scalar queue so the first
            # projection group can start as soon as both land.
            nc.scalar.dma_start(wq_sb[:, :, 0:128], wq_e[:, :, 0:128])
            nc.scalar.dma_start(wq_sb[:, :, 128:256], wq_e[:, :, 128:256])
            nc.scalar.dma_start(wk_sb[:], wk_e[:])

            def load_x1(b, split_first=False):
                x1t = x1p.tile([128, KC, LS], bf16, tag="x1", name=f"x1_{b}")
                for t in range(NT):
                    tsl = slice(t * 512, (t + 1) * 512)
                    if t == 0 and split_first:
                        for kc in range(KC):
                            nc.sync.dma_start(x1t[:, kc:kc + 1, 0:512],
                                              x1_e[b, 0, :, kc:kc + 1, :])
                    elif t == NT - 1 and split_first:
                        # keep the critical path on two queues: last tile
                        # rides the scalar queue (after wq/wk, below)
                        pass
                    else:
                        nc.sync.dma_start(x1t[:, :, tsl], x1_e[b, t])
                return x1t

            x1_next = load_x1(0, split_first=True)
            nc.scalar.dma_start(x1_next[:, :, (NT - 1) * 512:NT * 512],
                                x1_e[0, NT - 1])
            wv_sb = constp.tile([128, KC, CR], bf16, tag="wv")
            nc.scalar.dma_start(wv_sb[:], wv_e[:])
            if has_bias:
                bia_sb = constp.tile([128, 2 * MC + 2 + OC], f32, tag="bia")
                nc.scalar.dma_start(bia_sb[:], bia_e[:])
                bq_sb = bia_sb[:, 0:MC]
                bk_sb = bia_sb[:, MC:2 * MC]
                bv_sb = bia_sb[:, 2 * MC:2 * MC + 2]
                bo_sb = bia_sb[:, 2 * MC + 2:]
            id_sb = constp.tile([128, 128], bf16, tag="id")
            nc.scalar.dma_start(id_sb[:], id_e[:])
            if has_mask:
                on_sb = constp.tile([2, 128], bf16, tag="on")
                nc.scalar.dma_start(on_sb[:], on_e[:])
            wo_sb = constp.tile([128, MC, C_IN], bf16, tag="wo")
            nc.scalar.dma_start(wo_sb[:], wo_e[:])

            x1_tiles = {0: x1_next}
            qk_tiles = {}
            vt_tiles = {}
            mdts = {}

            if has_mask:
                for b in range(B):
                    mdt = smp.tile([2, NSTEP, 256], bf16, tag="md", bufs=4,
                                   name=f"md_{b}")
                    nc.scalar.dma_start(mdt[:], md_e[b])
                    mdts[b] = mdt

            def qk_thunks(b):
                """16 thunks, each one (q|k, m, t) projection group + cast."""
                x1t = x1_tiles[b]
                q_sb = [[None] * NT for _ in range(MC)]
                k_sb = [[None] * NT for _ in range(MC)]
                qk_tiles[b] = (q_sb, k_sb)
                thunks = []

                def th(t, m, isq):
                    tsl = slice(t * 512, (t + 1) * 512)
                    msl = slice(m * 128, (m + 1) * 128)
                    w_sb = wq_sb if isq else wk_sb
                    ps = psA.tile([128, 512], f32, tag="psA",
                                  name=f"{'q' if isq else 'k'}ps_{b}_{t}_{m}")
                    for kc in range(KC):
                        nc.tensor.matmul(
                            ps[:], w_sb[:, kc, msl], x1t[:, kc, tsl],
                            start=(kc == 0), stop=(kc == KC - 1),
                        )
                    qt = qkp.tile([128, 512], bf16, tag="qk",
                                  name=f"{'q' if isq else 'k'}_{b}_{t}_{m}")
                    if isq:
                        if has_bias:
                            nc.scalar.activation(qt[:], ps[:], AF.Identity,
                                                 bias=bq_sb[:, m:m + 1])
                        else:
                            nc.scalar.copy(qt[:], ps[:])
                        q_sb[m][t] = qt
                    else:
                        if has_bias:
                            nc.vector.tensor_scalar_add(qt[:], ps[:], bk_sb[:, m:m + 1])
                        else:
                            nc.vector.tensor_copy(qt[:], ps[:])
                        k_sb[m][t] = qt

                for t in range(NT):
                    for m in range(MC):
                        thunks.append(lambda t=t, m=m: th(t, m, True))
                        thunks.append(lambda t=t, m=m: th(t, m, False))
                return thunks

            def v_group(b, u):
                """v^T projection for step u's two 128-l-chunks + cast."""
                x1t = x1_tiles[b]
                ps = psA.tile([128, 2, CR], f32, tag="psA", name=f"vps_{b}_{u}")
                for j in range(2):
                    psl = slice((2 * u + j) * 128, (2 * u + j + 1) * 128)
                    for kc in range(KC):
                        nc.tensor.matmul(
                            ps[:, j, :], x1t[:, kc, psl], wv_sb[:, kc, :],
                            start=(j == 0 and kc == 0),
                            stop=(j == 1 and kc == KC - 1),
                        )
                vt = vtp.tile([128, 2, CR], bf16, tag="vt", name=f"vt_{b}_{u}")
                if u % 2 == 0:
                    nc.scalar.copy(vt[:], ps[:])
                else:
                    nc.vector.tensor_copy(vt[:], ps[:])
                vt_tiles[b].append(vt)

            def attn_front(b, u):
                """scores + softmax for pairs (2u, 2u+1); returns probs."""
                q_sb, k_sb = qk_tiles[b]
                t, half = divmod(u, 2)
                w = half * 256
                sc = psS.tile([128, 2, 128], f32, tag="sc", name=f"sc_{b}_{u}")
                for u2 in range(2):
                    qsl = slice(w + u2 * 128, w + u2 * 128 + 128)
                    for m in range(MC):
                        last = u2 == 1 and m == MC - 1
                        nc.tensor.matmul(
                            sc[:, u2, :],
                            q_sb[m][t][:, qsl], k_sb[m][t][:, qsl],
                            start=(u2 == 0 and m == 0),
                            stop=(last and not has_mask),
                        )
                if has_mask:
                    nc.tensor.matmul(sc[:], on_sb[:], mdts[b][:, u, :],
                                     start=False, stop=True)

                esc = smp.tile([128, 2, 128], bf16, tag="esc", name=f"esc_{b}_{u}")
                nc.scalar.activation(esc[:], sc[:], AF.Exp)
                if not has_mask:
                    # last batch has no q/k feeds and is vector-bound, so
                    # its quadrant-zeroing rides the idle gpsimd instead
                    mseng = nc.gpsimd if b == B - 1 else nc.vector
                    mseng.memset(esc[0:64, :, 64:128], 0)
                    mseng.memset(esc[64:128, :, 0:64], 0)
                rs = smp.tile([128, 2], f32, tag="rs", name=f"rs_{b}_{u}")
                nc.vector.reduce_sum(out=rs[:], in_=esc[:], axis=mybir.AxisListType.X)
                rc = smp.tile([128, 2], f32, tag="rc", name=f"rc_{b}_{u}")
                nc.vector.reciprocal(rc[:], rs[:])
                pr = smp.tile([128, 2, 128], bf16, tag="pr", name=f"pr_{b}_{u}")
                nc.scalar.mul(pr[:, 0, :], esc[:, 0, :], rc[:, 0:1])
                if b == B - 1:
                    nc.scalar.mul(pr[:, 1, :], esc[:, 1, :], rc[:, 1:2])
                else:
                    nc.vector.tensor_scalar_mul(pr[:, 1, :], esc[:, 1, :], rc[:, 1:2])
                return pr

            def attn_back(b, u, pr, h_sb):
                """transpose + attn + relu for pairs (2u, 2u+1)."""
                t, half = divmod(u, 2)
                prT_ps = psT.tile([128, 256], bf16, tag="prT", name=f"prT_{b}_{u}")
                for u2 in range(2):
                    nc.tensor.transpose(
                        prT_ps[:, u2 * 128:(u2 + 1) * 128], pr[:, u2, :], id_sb[:],
                    )
                prT = smp.tile([128, 256], bf16, tag="prTs", name=f"prTs_{b}_{u}")
                nc.vector.tensor_copy(prT[:], prT_ps[:])
                at = psAt.tile([128, MC, 2, 128], f32, tag="at", name=f"at_{b}_{u}")
                for u2 in range(2):
                    for m in range(MC):
                        nc.tensor.matmul(
                            at[:, m, u2, :],
                            vt_tiles[b][u][:, u2, m * 128:(m + 1) * 128],
                            prT[:, u2 * 128:(u2 + 1) * 128],
                            start=True, stop=True,
                        )
                if has_bias:
                    for m in range(MC):
                        hout = h_sb[t][:, m, half * 2:half * 2 + 2, :]
                        nc.scalar.activation(hout, at[:, m, :, :], AF.Relu,
                                             bias=bv_sb[:, m:m + 1])
                elif b == B - 1:
                    # last batch has no q/k feeds: its steps are
                    # scalar-bound, so split the relu across both engines
                    nc.scalar.activation(
                        h_sb[t][:, 0, half * 2:half * 2 + 2, :],
                        at[:, 0, :, :], AF.Relu)
                    nc.vector.tensor_scalar_max(
                        h_sb[t][:, 1, half * 2:half * 2 + 2, :],
                        at[:, 1, :, :], 0.0)
                else:
                    nc.scalar.activation(
                        h_sb[t][:, :, half * 2:half * 2 + 2, :], at[:],
                        AF.Relu,
                    )

            def out_piece(b, t, o, h_sb, dma_eng=None, fine=False):
                """output projection + store for one (l-tile, o-chunk)."""
                osl = slice(o * 128, (o + 1) * 128)
                ps = psA.tile([128, 512], f32, tag="psA",
                              name=f"ops_{b}_{t}_{o}")
                for m in range(MC):
                    nc.tensor.matmul(
                        ps[:], wo_sb[:, m, osl], h_sb[t][:, m, :, :],
                        start=(m == 0), stop=(m == MC - 1),
                    )
                ost = outp.tile([128, 512], bf16, tag="ot",
                                name=f"ot_{b}_{t}_{o}")
                if fine and not has_bias:
                    # kernel-final pieces: half-width casts and stores on
                    # parallel engines to compress the drain chain
                    nc.scalar.copy(ost[:, 0:256], ps[:, 0:256])
                    nc.vector.tensor_copy(ost[:, 256:512], ps[:, 256:512])
                    nc.sync.dma_start(out_e[b, t, o, :, 0:256], ost[:, 0:256])
                    nc.scalar.dma_start(out_e[b, t, o, :, 256:512],
                                        ost[:, 256:512])
                    return
                if has_bias:
                    if o % 2 == 0:
                        nc.scalar.activation(ost[:], ps[:], AF.Identity,
                                             bias=bo_sb[:, o:o + 1])
                    else:
                        nc.vector.tensor_scalar_add(ost[:], ps[:],
                                                    bo_sb[:, o:o + 1])
                else:
                    if o % 2 == 0:
                        nc.scalar.copy(ost[:], ps[:])
                    else:
                        nc.vector.tensor_copy(ost[:], ps[:])
                (dma_eng or nc.sync).dma_start(out_e[b, t, o], ost[:])

            def attention(b, feeds):
                """8 attention steps for batch b, 3-stage software-pipelined,
                interleaved with next batch's q/k projection groups (feeds)
                and this batch's output pieces. Returns leftover pieces."""
                h_sb = [hp.tile([128, MC, 4, 128], bf16, tag="h",
                                name=f"h_{b}_{t}") for t in range(NT)]
                pieces = []
                fi = [0]

                def feed():
                    if fi[0] < len(feeds):
                        feeds[fi[0]]()
                        fi[0] += 1

                def after_bk(bk):
                    # h[t] complete once bk = 2t+2; enqueue pieces one BK
                    # later so the relu has settled.
                    if bk >= 3 and bk % 2 == 1:
                        pieces.extend(((bk - 3) // 2, o) for o in range(OC))
                    # the last batch has no q/k feeds, so drain pieces faster
                    for _ in range(3 if b == B - 1 else 2):
                        if pieces:
                            t, o = pieces.pop(0)
                            out_piece(b, t, o, h_sb)

                prs = {}
                bk = 0
                for u in range(NSTEP):
                    prs[u] = attn_front(b, u)
                    feed()
                    if u == 0 and b + 2 < B:
                        x1_tiles[b + 2] = load_x1(b + 2)
                    if u >= 2:
                        attn_back(b, bk, prs.pop(bk), h_sb)
                        bk += 1
                        after_bk(bk)
                    feed()
                while bk < NSTEP:
                    attn_back(b, bk, prs.pop(bk), h_sb)
                    bk += 1
                    after_bk(bk)
                    feed()
                return [(b, t, o, h_sb) for (t, o) in pieces] + \
                       [(b, NT - 1, o, h_sb) for o in range(OC)]

            # batch 0 q/k+v up front; thereafter batch b+1's q/k rides inside
            # batch b's attention phase and its v phase follows, interleaved
            # with batch b's leftover output pieces.
            for thunk in qk_thunks(0):
                thunk()
            x1_tiles[1] = load_x1(1)
            vt_tiles[0] = []
            for u in range(NSTEP):
                v_group(0, u)

            for b in range(B):
                feeds = qk_thunks(b + 1) if b + 1 < B else []
                leftovers = attention(b, feeds)
                if b + 1 < B:
                    vt_tiles[b + 1] = []
                    for u in range(NSTEP):
                        v_group(b + 1, u)
                        if leftovers:
                            lb, lt, lo, lh = leftovers.pop(0)
                            out_piece(lb, lt, lo, lh)
                else:
                    for lb, lt, lo, lh in leftovers:
                        out_piece(lb, lt, lo, lh,
                                  dma_eng=nc.scalar if lo % 2 else nc.sync,
                                  fine=(lo >= OC - 2))

    nc.compile()
    return nc


def _get_graph(has_bias, has_mask):
    key = ("nc", has_bias, has_mask)
    if key not in _CACHE:
        _CACHE[key] = _build_graph(has_bias, has_mask)
    return _CACHE[key]


def _make_in_maps(inputs, has_bias, has_mask):
    x1 = np.asarray(inputs["x1"])
    mask = np.asarray(inputs["mask"])
    Wq, bq = np.asarray(inputs["Wq"]), np.asarray(inputs["bq"])
    Wk, bk = np.asarray(inputs["Wk"]), np.asarray(inputs["bk"])
    Wv, bv = np.asarray(inputs["Wv"]), np.asarray(inputs["bv"])
    Wo, bo = np.asarray(inputs["Wo"]), np.asarray(inputs["bo"])

    bf16 = ml_dtypes.bfloat16

    def warr(w):
        # [C_IN, C] -> [128, KC, C] with row (k*128+p) -> [p, k, :]
        cin, c = w.shape
        return np.ascontiguousarray(
            w.reshape(cin // 128, 128, c).transpose(1, 0, 2)).astype(bf16)

    wq = warr(np.ascontiguousarray(Wq.T) * np.float32(QSCALE))
    wk = warr(np.ascontiguousarray(Wk.T))
    wv = warr(np.ascontiguousarray(Wv.T))
    wo = warr(np.ascontiguousarray(Wo.T))

    ident = np.eye(128, dtype=bf16)
    shared = {
        "wq": wq, "wk": wk, "wv": wv, "wo": wo,
        "ident": ident,
    }
    if has_mask:
        onesbd = np.zeros((2, 128), dtype=bf16)
        onesbd[0, :64] = 1
        onesbd[1, 64:] = 1
        shared["onesbd"] = onesbd
    if has_bias:
        biases = np.concatenate([
            (bq * QSCALE).reshape(MC, 128).T,
            bk.reshape(MC, 128).T,
            bv.reshape(MC, 128).T,
            bo.reshape(OC, 128).T,
        ], axis=1).astype(np.float32)
        shared["biases"] = np.ascontiguousarray(biases)

    x1b = x1.astype(bf16)
    if has_mask:
        madd = np.where(mask[:, 0, :] == 0, np.float32(NEGM), np.float32(0.0))

    in_maps = []
    for c in range(N_CORES):
        sl = slice(c * LS, (c + 1) * LS)
        # [B, C_IN, LS] -> [B, NT, 128, KC, 512]
        x1s = np.ascontiguousarray(
            x1b[:, :, sl].reshape(B, KC, 128, NT, 512).transpose(0, 3, 2, 1, 4))
        m = {"x1s": x1s, **shared}
        if has_mask:
            m4 = madd[:, sl].reshape(B, NSTEP, 2, 2, 64)
            md2 = np.full((B, 2, NSTEP, 2, 2, 64), NEGM, np.float32)
            md2[:, 0, :, :, 0, :] = m4[:, :, :, 0, :]
            md2[:, 1, :, :, 1, :] = m4[:, :, :, 1, :]
            m["madd2"] = md2.reshape(B, 2, NSTEP, 256).astype(bf16)
        in_maps.append(m)
    return in_maps


def kernel(**inputs):
    from concourse.bass_utils import run_bass_kernel_spmd

    has_bias = any(
        np.asarray(inputs[k]).any() for k in ("bq", "bk", "bv", "bo"))
    has_mask = not np.all(np.asarray(inputs["mask"]) == 1)
    nc = _get_graph(has_bias, has_mask)
    in_maps = _make_in_maps(inputs, has_bias, has_mask)
    res = run_bass_kernel_spmd(nc, in_maps, core_ids=list(range(N_CORES)))
    _CACHE["last_results"] = res
    outs = []
    for i in range(N_CORES):
        o = np.asarray(res.results[i]["out"]).astype(np.float32)
        # [B, NT, OC, 128, 512] -> [B, OC*128, NT*512]
        o = o.transpose(0, 2, 3, 1, 4).reshape(B, C_IN, LS)
        outs.append(o)
    return np.concatenate(outs, axis=2)


# revision 41
# speedup vs baseline: 1.0124x; 1.0124x over previous
"""Trainium2 Bass kernel for blocked (non-overlapping window) attention.

Reference computation (per batch b):
    q = Wq @ x1 ; k = Wk @ x1 ; v = Wv @ x1          (1x1 convs, biases)
    split L into blocks of 64; per block: softmax((q^T k)/sqrt(C) masked) @ v^T
    h = relu(attn); out = Wo @ h + bo

Sharding: sequence-parallel over L: each of 8 cores gets a contiguous
L/8 = 2048 slice for all 4 batches; small conv weights replicated.

Numerics: matmuls bf16, f32 psum accumulation. 1/sqrt(C) is folded into
Wq host-side (exact: power of two). Two 64-blocks are processed per
128-partition pair; two pairs (256 l-positions) per attention step.
Softmax skips max-subtraction; the cross-block quadrants of exp(scores)
are zeroed by two vector memsets before the row-sum (no additive mask
matmul needed when the user mask is all ones). Output is stored bf16
and upconverted host-side.

Fast path assumes the zero biases / all-ones mask that setup_inputs()
produces; general inputs fall back to a variant graph with bias
epilogues and per-(batch,step) additive -30000 mask matmuls.

Schedule notes (the big wins over a naive phase-by-phase loop):
 - batch b+1's q/k projection groups (pure PE work) are interleaved
   into batch b's attention steps, so the scalar/vector-heavy softmax
   epilogues always overlap PE-heavy projection segments;
 - 3-stage software pipeline inside attention: transpose(u) trails
   scores(u) by two PE packets, hiding the cross-engine
   exp->reduce->recip->normalize chain;
 - output projection is emitted as per-(tile, o-chunk) pieces
   sprinkled between steps, stores stream on the sync queue
   immediately (bf16, one descriptor per piece);
 - epilogue casts alternate scalar/vector (gpsimd has no PSUM port and
   its tensor ops are ~8x slower - only DMA issue is cheap there);
 - psum banks: 2 shared (q/k/v/out) + 3 scores (pipeline depth) +
   1 transpose + 2 attention = 8 exactly.
"""

import sys

sys.path.insert(0, "/opt/trn_rl_repo")

import numpy as np
import ml_dtypes

B = 4
C_IN = 512
L = 16384
CR = 256           # reduced (q/k/v) channels
BL = 64            # attention block
N_CORES = 8
LS = L // N_CORES  # 2048 per-core sequence shard
NT = LS // 512     # 4 free-dim tiles of 512
KC = C_IN // 128   # 4 contraction chunks for q/k/v projections
MC = CR // 128     # 2 chunks of reduced channels
OC = C_IN // 128   # 4 chunks of output channels
NPAIR = LS // 128  # 16 block-pairs per batch per core
NSTEP = NPAIR // 2 # 8 attention steps (2 pairs each) per batch
NEGM = -30000.0
QSCALE = 1.0 / 16.0  # 1/sqrt(C_RED)

_CACHE = {}


def _build_graph(has_bias, has_mask):
    import concourse.bass as bass
    import concourse.tile as tile
    from concourse import bacc, mybir

    f32 = mybir.dt.float32
    bf16 = mybir.dt.bfloat16
    AF = mybir.ActivationFunctionType

    nc = bacc.Bacc(None, target_bir_lowering=False)

    x1_e = nc.declare_dram_parameter("x1s", [B, NT, 128, KC, 512], bf16, isOutput=False)
    wq_e = nc.declare_dram_parameter("wq", [128, KC, CR], bf16, isOutput=False)
    wk_e = nc.declare_dram_parameter("wk", [128, KC, CR], bf16, isOutput=False)
    wv_e = nc.declare_dram_parameter("wv", [128, KC, CR], bf16, isOutput=False)
    wo_e = nc.declare_dram_parameter("wo", [128, MC, C_IN], bf16, isOutput=False)
    id_e = nc.declare_dram_parameter("ident", [128, 128], bf16, isOutput=False)
    if has_mask:
        on_e = nc.declare_dram_parameter("onesbd", [2, 128], bf16, isOutput=False)
        md_e = nc.declare_dram_parameter("madd2", [B, 2, NSTEP, 256], bf16, isOutput=False)
    if has_bias:
        # packed per-channel biases: cols [0:2]=bq*QSCALE, [2:4]=bk, [4:6]=bv, [6:10]=bo
        bia_e = nc.declare_dram_parameter("biases", [128, 2 * MC + 2 + OC], f32, isOutput=False)
    out_e = nc.declare_dram_parameter("out", [B, NT, OC, 128, 512], bf16, isOutput=True)

    PS = bass.MemorySpace.PSUM

    with tile.TileContext(nc) as tc:
        with (
            tc.tile_pool(name="const", bufs=1) as constp,
            tc.tile_pool(name="x1p", bufs=2) as x1p,
            tc.tile_pool(name="qkp", bufs=32) as qkp,
            tc.tile_pool(name="vtp", bufs=16) as vtp,
            tc.tile_pool(name="hp", bufs=6) as hp,
            tc.tile_pool(name="outp", bufs=4) as outp,
            tc.tile_pool(name="smp", bufs=4) as smp,
            tc.tile_pool(name="psA", bufs=3, space=PS) as psA,
            tc.tile_pool(name="psS", bufs=2, space=PS) as psS,
            tc.tile_pool(name="psT", bufs=1, space=PS) as psT,
            tc.tile_pool(name="psAt", bufs=2, space=PS) as psAt,
        ):
            # ---- constants (DMA'd on the scalar queue, ordered by first use;
            # x1 tile loads ride the sync queue in parallel) ----
            wq_sb = constp.tile([128, KC, CR], bf16, tag="wq")
            wk_sb = constp.tile([128, KC, CR], bf16, tag="wk")
            # critical-path order: x1 t0 halves lead the sync queue (below);
            # wq's first m-chunk leads the scalar queue so the first
            # projection group can start as soon as both land.
            nc.scalar.dma_start(wq_sb[:, :, 0:128], wq_e[:, :, 0:128])
            nc.scalar.dma_start(wq_sb[:, :, 128:256], wq_e[:, :, 128:256])
            nc.scalar.dma_start(wk_sb[:], wk_e[:])

            def load_x1(b, split_first=False):
                x1t = x1p.tile([128, KC, LS], bf16, tag="x1", name=f"x1_{b}")
                for t in range(NT):
                    tsl = slice(t * 512, (t + 1) * 512)
                    if t == 0 and split_first:
                        for kc in range(KC):
                            nc.sync.dma_start(x1t[:, kc:kc + 1, 0:512],
                                              x1_e[b, 0, :, kc:kc + 1, :])
                    elif t == NT - 1 and split_first:
                        # keep the critical path on two queues: last tile
                        # rides the scalar queue (after wq/wk, below)
                        pass
                    else:
                        nc.sync.dma_start(x1t[:, :, tsl], x1_e[b, t])
                return x1t

            x1_next = load_x1(0, split_first=True)
            nc.scalar.dma_start(x1_next[:, :, (NT - 1) * 512:NT * 512],
                                x1_e[0, NT - 1])
            wv_sb = constp.tile([128, KC, CR], bf16, tag="wv")
            nc.scalar.dma_start(wv_sb[:], wv_e[:])
            if has_bias:
                bia_sb = constp.tile([128, 2 * MC + 2 + OC], f32, tag="bia")
                nc.scalar.dma_start(bia_sb[:], bia_e[:])
                bq_sb = bia_sb[:, 0:MC]
                bk_sb = bia_sb[:, MC:2 * MC]
                bv_sb = bia_sb[:, 2 * MC:2 * MC + 2]
                bo_sb = bia_sb[:, 2 * MC + 2:]
            id_sb = constp.tile([128, 128], bf16, tag="id")
            nc.scalar.dma_start(id_sb[:], id_e[:])
            if has_mask:
                on_sb = constp.tile([2, 128], bf16, tag="on")
                nc.scalar.dma_start(on_sb[:], on_e[:])
            wo_sb = constp.tile([128, MC, C_IN], bf16, tag="wo")
            nc.scalar.dma_start(wo_sb[:], wo_e[:])

            x1_tiles = {0: x1_next}
            qk_tiles = {}
            vt_tiles = {}
            mdts = {}

            if has_mask:
                for b in range(B):
                    mdt = smp.tile([2, NSTEP, 256], bf16, tag="md", bufs=4,
                                   name=f"md_{b}")
                    nc.scalar.dma_start(mdt[:], md_e[b])
                    mdts[b] = mdt

            def qk_thunks(b):
                """16 thunks, each one (q|k, m, t) projection group + cast."""
                x1t = x1_tiles[b]
                q_sb = [[None] * NT for _ in range(MC)]
                k_sb = [[None] * NT for _ in range(MC)]
                qk_tiles[b] = (q_sb, k_sb)
                thunks = []

                def th(t, m, isq):
                    tsl = slice(t * 512, (t + 1) * 512)
                    msl = slice(m * 128, (m + 1) * 128)
                    w_sb = wq_sb if isq else wk_sb
                    ps = psA.tile([128, 512], f32, tag="psA",
                                  name=f"{'q' if isq else 'k'}ps_{b}_{t}_{m}")
                    for kc in range(KC):
                        nc.tensor.matmul(
                            ps[:], w_sb[:, kc, msl], x1t[:, kc, tsl],
                            start=(kc == 0), stop=(kc == KC - 1),
                        )
                    qt = qkp.tile([128, 512], bf16, tag="qk",
                                  name=f"{'q' if isq else 'k'}_{b}_{t}_{m}")
                    if isq:
                        if has_bias:
                            nc.scalar.activation(qt[:], ps[:], AF.Identity,
                                                 bias=bq_sb[:, m:m + 1])
                        else:
                            nc.scalar.copy(qt[:], ps[:])
                        q_sb[m][t] = qt
                    else:
                        if has_bias:
                            nc.vector.tensor_scalar_add(qt[:], ps[:], bk_sb[:, m:m + 1])
                        else:
                            nc.vector.tensor_copy(qt[:], ps[:])
                        k_sb[m][t] = qt

                for t in range(NT):
                    for m in range(MC):
                        thunks.append(lambda t=t, m=m: th(t, m, True))
                        thunks.append(lambda t=t, m=m: th(t, m, False))
                return thunks

            def v_group(b, u):
                """v^T projection for step u's two 128-l-chunks + cast."""
                x1t = x1_tiles[b]
                ps = psA.tile([128, 2, CR], f32, tag="psA", name=f"vps_{b}_{u}")
                for j in range(2):
                    psl = slice((2 * u + j) * 128, (2 * u + j + 1) * 128)
                    for kc in range(KC):
                        nc.tensor.matmul(
                            ps[:, j, :], x1t[:, kc, psl], wv_sb[:, kc, :],
                            start=(j == 0 and kc == 0),
                            stop=(j == 1 and kc == KC - 1),
                        )
                vt = vtp.tile([128, 2, CR], bf16, tag="vt", name=f"vt_{b}_{u}")
                if u % 2 == 0:
                    nc.scalar.copy(vt[:], ps[:])
                else:
                    nc.vector.tensor_copy(vt[:], ps[:])
                vt_tiles[b].append(vt)

            def attn_front(b, u):
                """scores + softmax for pairs (2u, 2u+1); returns probs."""
                q_sb, k_sb = qk_tiles[b]
                t, half = divmod(u, 2)
                w = half * 256
                sc = psS.tile([128, 2, 128], f32, tag="sc", name=f"sc_{b}_{u}")
                for u2 in range(2):
                    qsl = slice(w + u2 * 128, w + u2 * 128 + 128)
                    for m in range(MC):
                        last = u2 == 1 and m == MC - 1
                        nc.tensor.matmul(
                            sc[:, u2, :],
                            q_sb[m][t][:, qsl], k_sb[m][t][:, qsl],
                            start=(u2 == 0 and m == 0),
                            stop=(last and not has_mask),
                        )
                if has_mask:
                    nc.tensor.matmul(sc[:], on_sb[:], mdts[b][:, u, :],
                                     start=False, stop=True)

                esc = smp.tile([128, 2, 128], bf16, tag="esc", name=f"esc_{b}_{u}")
                nc.scalar.activation(esc[:], sc[:], AF.Exp)
                if not has_mask:
                    # last batch has no q/k feeds and is vector-bound, so
                    # its quadrant-zeroing rides the idle gpsimd instead
                    mseng = nc.gpsimd if b == B - 1 else nc.vector
                    mseng.memset(esc[0:64, :, 64:128], 0)
                    mseng.memset(esc[64:128, :, 0:64], 0)
                rs = smp.tile([128, 2], f32, tag="rs", name=f"rs_{b}_{u}")
                nc.vector.reduce_sum(out=rs[:], in_=esc[:], axis=mybir.AxisListType.X)
                rc = smp.tile([128, 2], f32, tag="rc", name=f"rc_{b}_{u}")
                nc.vector.reciprocal(rc[:], rs[:])
                pr = smp.tile([128, 2, 128], bf16, tag="pr", name=f"pr_{b}_{u}")
                nc.scalar.mul(pr[:, 0, :], esc[:, 0, :], rc[:, 0:1])
                if b == B - 1:
                    nc.scalar.mul(pr[:, 1, :], esc[:, 1, :], rc[:, 1:2])
                else:
                    nc.vector.tensor_scalar_mul(pr[:, 1, :], esc[:, 1, :], rc[:, 1:2])
                return pr

            def attn_back(b, u, pr, h_sb):
                """transpose + attn + relu for pairs (2u, 2u+1)."""
                t, half = divmod(u, 2)
                prT_ps = psT.tile([128, 256], bf16, tag="prT", name=f"prT_{b}_{u}")
                for u2 in range(2):
                    nc.tensor.transpose(
                        prT_ps[:, u2 * 128:(u2 + 1) * 128], pr[:, u2, :], id_sb[:],
                    )
                prT = smp.tile([128, 256], bf16, tag="prTs", name=f"prTs_{b}_{u}")
                nc.vector.tensor_copy(prT[:], prT_ps[:])
                at = psAt.tile([128, MC, 2, 128], f32, tag="at", name=f"at_{b}_{u}")
                for u2 in range(2):
                    for m in range(MC):
                        nc.tensor.matmul(
                            at[:, m, u2, :],
                            vt_tiles[b][u][:, u2, m * 128:(m + 1) * 128],
                            prT[:, u2 * 128:(u2 + 1) * 128],
                            start=True, stop=True,
                        )
                if has_bias:
                    for m in range(MC):
                        hout = h_sb[t][:, m, half * 2:half * 2 + 2, :]
                        nc.scalar.activation(hout, at[:, m, :, :], AF.Relu,
                                             bias=bv_sb[:, m:m + 1])
                elif b == B - 1:
                    # last batch has no q/k feeds: its steps are
                    # scalar-bound, so split the relu across both engines
                    nc.scalar.activation(
                        h_sb[t][:, 0, half * 2:half * 2 + 2, :],
                        at[:, 0, :, :], AF.Relu)
                    nc.vector.tensor_scalar_max(
                        h_sb[t][:, 1, half * 2:half * 2 + 2, :],
                        at[:, 1, :, :], 0.0)
                else:
                    nc.scalar.activation(
                        h_sb[t][:, :, half * 2:half * 2 + 2, :], at[:],
                        AF.Relu,
                    )

            def out_piece(b, t, o, h_sb, dma_eng=None):
                """output projection + store for one (l-tile, o-chunk)."""
                osl = slice(o * 128, (o + 1) * 128)
                ps = psA.tile([128, 512], f32, tag="psA",
                              name=f"ops_{b}_{t}_{o}")
                for m in range(MC):
                    nc.tensor.matmul(
                        ps[:], wo_sb[:, m, osl], h_sb[t][:, m, :, :],
                        start=(m == 0), stop=(m == MC - 1),
                    )
                ost = outp.tile([128, 512], bf16, tag="ot",
                                name=f"ot_{b}_{t}_{o}")
                if has_bias:
                    if o % 2 == 0:
                        nc.scalar.activation(ost[:], ps[:], AF.Identity,
                                             bias=bo_sb[:, o:o + 1])
                    else:
                        nc.vector.tensor_scalar_add(ost[:], ps[:],
                                                    bo_sb[:, o:o + 1])
                else:
                    if o % 2 == 0:
                        nc.scalar.copy(ost[:], ps[:])
                    else:
                        nc.vector.tensor_copy(ost[:], ps[:])
                (dma_eng or nc.sync).dma_start(out_e[b, t, o], ost[:])

            def attention(b, feeds):
                """8 attention steps for batch b, 3-stage software-pipelined,
                interleaved with next batch's q/k projection groups (feeds)
                and this batch's output pieces. Returns leftover pieces."""
                h_sb = [hp.tile([128, MC, 4, 128], bf16, tag="h",
                                name=f"h_{b}_{t}") for t in range(NT)]
                pieces = []
                fi = [0]

                def feed():
                    if fi[0] < len(feeds):
                        feeds[fi[0]]()
                        fi[0] += 1

                def after_bk(bk):
                    # h[t] complete once bk = 2t+2; enqueue pieces one BK
                    # later so the relu has settled.
                    if bk >= 3 and bk % 2 == 1:
                        pieces.extend(((bk - 3) // 2, o) for o in range(OC))
                    # the last batch has no q/k feeds, so drain pieces faster
                    for _ in range(3 if b == B - 1 else 2):
                        if pieces:
                            t, o = pieces.pop(0)
                            out_piece(b, t, o, h_sb)

                prs = {}
                bk = 0
                for u in range(NSTEP):
                    prs[u] = attn_front(b, u)
                    feed()
                    if u == 0 and b + 2 < B:
                        x1_tiles[b + 2] = load_x1(b + 2)
                    if u >= 2:
                        attn_back(b, bk, prs.pop(bk), h_sb)
                        bk += 1
                        after_bk(bk)
                    feed()
                while bk < NSTEP:
                    attn_back(b, bk, prs.pop(bk), h_sb)
                    bk += 1
                    after_bk(bk)
                    feed()
                return [(b, t, o, h_sb) for (t, o) in pieces] + \
                       [(b, NT - 1, o, h_sb) for o in range(OC)]

            # batch 0 q/k+v up front; thereafter batch b+1's q/k rides inside
            # batch b's attention phase and its v phase follows, interleaved
            # with batch b's leftover output pieces.
            for thunk in qk_thunks(0):
                thunk()
            x1_tiles[1] = load_x1(1)
            vt_tiles[0] = []
            for u in range(NSTEP):
                v_group(0, u)

            for b in range(B):
                feeds = qk_thunks(b + 1) if b + 1 < B else []
                leftovers = attention(b, feeds)
                if b + 1 < B:
                    vt_tiles[b + 1] = []
                    for u in range(NSTEP):
                        v_group(b + 1, u)
                        if leftovers:
                            lb, lt, lo, lh = leftovers.pop(0)
                            out_piece(lb, lt, lo, lh)
                else:
                    for lb, lt, lo, lh in leftovers:
                        out_piece(lb, lt, lo, lh,
                                  dma_eng=nc.scalar if lo % 2 else nc.sync)

    nc.compile()
    return nc


def _get_graph(has_bias, has_mask):
    key = ("nc", has_bias, has_mask)
    if key not in _CACHE:
        _CACHE[key] = _build_graph(has_bias, has_mask)
    return _CACHE[key]


def _make_in_maps(inputs, has_bias, has_mask):
    x1 = np.asarray(inputs["x1"])
    mask = np.asarray(inputs["mask"])
    Wq, bq = np.asarray(inputs["Wq"]), np.asarray(inputs["bq"])
    Wk, bk = np.asarray(inputs["Wk"]), np.asarray(inputs["bk"])
    Wv, bv = np.asarray(inputs["Wv"]), np.asarray(inputs["bv"])
    Wo, bo = np.asarray(inputs["Wo"]), np.asarray(inputs["bo"])

    bf16 = ml_dtypes.bfloat16

    def warr(w):
        # [C_IN, C] -> [128, KC, C] with row (k*128+p) -> [p, k, :]
        cin, c = w.shape
        return np.ascontiguousarray(
            w.reshape(cin // 128, 128, c).transpose(1, 0, 2)).astype(bf16)

    wq = warr(np.ascontiguousarray(Wq.T) * np.float32(QSCALE))
    wk = warr(np.ascontiguousarray(Wk.T))
    wv = warr(np.ascontiguousarray(Wv.T))
    wo = warr(np.ascontiguousarray(Wo.T))

    ident = np.eye(128, dtype=bf16)
    shared = {
        "wq": wq, "wk": wk, "wv": wv, "wo": wo,
        "ident": ident,
    }
    if has_mask:
        onesbd = np.zeros((2, 128), dtype=bf16)
        onesbd[0, :64] = 1
        onesbd[1, 64:] = 1
        shared["onesbd"] = onesbd
    if has_bias:
        biases = np.concatenate([
            (bq * QSCALE).reshape(MC, 128).T,
            bk.reshape(MC, 128).T,
            bv.reshape(MC, 128).T,
            bo.reshape(OC, 128).T,
        ], axis=1).astype(np.float32)
        shared["biases"] = np.ascontiguousarray(biases)

    x1b = x1.astype(bf16)
    if has_mask:
        madd = np.where(mask[:, 0, :] == 0, np.float32(NEGM), np.float32(0.0))

    in_maps = []
    for c in range(N_CORES):
        sl = slice(c * LS, (c + 1) * LS)
        # [B, C_IN, LS] -> [B, NT, 128, KC, 512]
        x1s = np.ascontiguousarray(
            x1b[:, :, sl].reshape(B, KC, 128, NT, 512).transpose(0, 3, 2, 1, 4))
        m = {"x1s": x1s, **shared}
        if has_mask:
            m4 = madd[:, sl].reshape(B, NSTEP, 2, 2, 64)
            md2 = np.full((B, 2, NSTEP, 2, 2, 64), NEGM, np.float32)
            md2[:, 0, :, :, 0, :] = m4[:, :, :, 0, :]
            md2[:, 1, :, :, 1, :] = m4[:, :, :, 1, :]
            m["madd2"] = md2.reshape(B, 2, NSTEP, 256).astype(bf16)
        in_maps.append(m)
    return in_maps


def kernel(**inputs):
    from concourse.bass_utils import run_bass_kernel_spmd

    has_bias = any(
        np.asarray(inputs[k]).any() for k in ("bq", "bk", "bv", "bo"))
    has_mask = not np.all(np.asarray(inputs["mask"]) == 1)
    nc = _get_graph(has_bias, has_mask)
    in_maps = _make_in_maps(inputs, has_bias, has_mask)
    res = run_bass_kernel_spmd(nc, in_maps, core_ids=list(range(N_CORES)))
    _CACHE["last_results"] = res
    outs = []
    for i in range(N_CORES):
        o = np.asarray(res.results[i]["out"]).astype(np.float32)
        # [B, NT, OC, 128, 512] -> [B, OC*128, NT*512]
        o = o.transpose(0, 2, 3, 1, 4).reshape(B, C_IN, LS)
        outs.append(o)
    return np.concatenate(outs, axis=2)
